# revision 1
# baseline (speedup 1.0000x reference)
"""Trainium2 Bass kernel for nn_NumDualDescriptorAB (sliding-window descriptor).

Reference computation:
    X = vec_seq @ M.T                       # [S, m]
    T[w] = mean_{r<rank} X[w+r]             # sliding window mean, W = S-rank+1
    j = w % L
    scalar[w] = Bbasis[j] . T[w]
    out[w]    = Acoeff.T[j] * scalar[w]

Algebraic rewrite (matmul is linear, dot distributes over the window sum):
    C = Bbasis @ M / rank                   # [L, m]  tiny - host precompute
    P[w] = sum_{r<rank} vec_seq[w+r]        # window *sum* of raw input rows
    scalar[w] = C[j] . P[w]
    out[w]    = Acoeff.T[j] * scalar[w]

bf16 end-to-end on the DMA path (tolerance 2e-2; this gives ~4e-3 l2):
  - input rounded to bf16 and block-swizzled on host so one 1MB DMA
    delivers NB=8 perfectly laid out [128rows, 512] chunks
  - PE: banded 0/1 matmuls compute window sums P into PSUM f32; the
    weights are fp8-e4m3 (0/1 exact, halves the first const DMA); per
    block a main sweep fills the 8 PSUM banks, then each halo matmul
    closes its bank with the fused STT + ACT chained right behind it
  - DVE: fused scalar_tensor_tensor gives scalar[w] = C[j].P[w] in one
    pass (mult + free-axis reduce; tensor_reduce is always 1x here, so
    the fusion halves DVE time)
  - ACT: activation(Copy, scale=scalar) broadcasts Acoeff.T rows as bf16
  - loads ride the SP HWDGE ring (block 0 per-chunk for a short ramp),
    half-block stores the GPSIMD SWDGE ring (per-chunk on the last block
    for a short tail flush), consts the ACT HWDGE ring
Sharded across 8 cores along the window axis; halo handled by a rank-1
row overlap staged host-side (no collectives).

Measured ~155-163us (thermal-state dependent) vs 299us baseline; the body
is PE-issue-rate-bound at 256 matmuls x ~537ns (512 free cycles + ~128
cycles LDWEIGHTS/drain at the power-clamped 1.2GHz PE clock), stall-free.
"""

import numpy as np

import concourse.bacc as bacc
import concourse.bass as bass  # noqa: F401  (AP types etc.)
import concourse.mybir as mybir
import concourse.tile as tile
from concourse.bass_utils import run_bass_kernel_spmd

N_CORES = 8
M_DIM = 512  # vector dim m (= free dim of every tile)
L_DIM = 512  # number of basis rows; window w uses row w % L_DIM
SEQ = 131072
CHUNK = 128  # rows per chunk == windows per psum tile
NB = 8  # chunks per DMA block (1MB bf16 transfers)
BLK = NB * M_DIM  # free-dim elems per block tile

_NC_CACHE = {}
_LAST_RESULTS = None  # BassKernelResults of the most recent run (for test.py)


def build_nc(nblk: int, rank: int) -> bass.Bass:
    f32 = mybir.dt.float32
    bf16 = mybir.dt.bfloat16
    halo = rank - 1

    # Bacc (not raw Bass): its compile() pipeline splits multi-wait
    # instructions (TRN2 allows 1 sync wait per instruction) via
    # generate_event_semaphores; raw Bass programs fail walrus codegen.
    nc = bacc.Bacc()
    v_d = nc.dram_tensor("v", [nblk, CHUNK, BLK], bf16, kind="ExternalInput")
    c_d = nc.dram_tensor("cmat", [4, CHUNK, M_DIM], bf16, kind="ExternalInput")
    a_d = nc.dram_tensor("amat", [4, CHUNK, M_DIM], bf16, kind="ExternalInput")
    fp8 = mybir.dt.float8e4
    w1_d = nc.dram_tensor("w1", [CHUNK, CHUNK], fp8, kind="ExternalInput")
    if halo > 0:
        w2_d = nc.dram_tensor("w2", [halo, CHUNK], fp8, kind="ExternalInput")
        vh_d = nc.dram_tensor("vh", [halo, M_DIM], bf16, kind="ExternalInput")
    o_d = nc.dram_tensor("o", [nblk, CHUNK, BLK], bf16, kind="ExternalOutput")

    mult = mybir.AluOpType.mult
    copy_f = mybir.ActivationFunctionType.Copy

    with tile.TileContext(nc) as tc:
        with (
            tc.tile_pool(name="consts", bufs=1) as consts,
            tc.tile_pool(name="blocks", bufs=7) as blocks,
            tc.tile_pool(name="outs", bufs=4) as outs,
            tc.tile_pool(name="psump", bufs=8, space="PSUM") as psump,
            tc.tile_pool(name="work", bufs=8) as work,
        ):
            c4 = consts.tile([CHUNK, 4, M_DIM], bf16, tag="c4")
            a4 = consts.tile([CHUNK, 4, M_DIM], bf16, tag="a4")
            w1t = consts.tile([CHUNK, CHUNK], fp8, tag="w1")
            # consts ride the ACT (scalar) HWDGE ring so the SP ring can
            # start streaming input blocks immediately. Weights first: the
            # first matmul only needs w1 + block0; c4/a4 aren't read until
            # the first STT/ACT several us later.
            nc.scalar.dma_start(out=w1t, in_=w1_d[:])
            if halo > 0:
                w2t = consts.tile([halo, CHUNK], fp8, tag="w2")
                nc.scalar.dma_start(out=w2t, in_=w2_d[:])
                vht = consts.tile([halo, M_DIM], bf16, tag="vh")
                nc.scalar.dma_start(out=vht, in_=vh_d[:])
            for h in range(4):
                nc.scalar.dma_start(out=c4[:, h, :], in_=c_d[h])
                nc.scalar.dma_start(out=a4[:, h, :], in_=a_d[h])

            def load_block(b):
                vt = blocks.tile([CHUNK, BLK], bf16, tag="vblk")
                if b == 0:
                    # Per-chunk loads for block 0 so the first matmul can
                    # start as soon as its 128KB slice lands (shorter ramp).
                    for c in range(NB):
                        nc.sync.dma_start(
                            out=vt[:, c * M_DIM : (c + 1) * M_DIM],
                            in_=v_d[b][:, c * M_DIM : (c + 1) * M_DIM],
                        )
                else:
                    nc.sync.dma_start(out=vt, in_=v_d[b])
                return vt

            # Software-pipelined prefetch: issue block DMAs PF blocks ahead
            # in program order so matmul waits are pre-satisfied.
            PF = 4
            vts = {}
            for b in range(min(PF, nblk)):
                vts[b] = load_block(b)
            for b in range(nblk):
                pf = b + PF
                if pf < nblk and pf not in vts:
                    vts[pf] = load_block(pf)
                vt = vts[b]
                ot = outs.tile([CHUNK, BLK], bf16, tag="oblk")
                pss = [
                    psump.tile([CHUNK, M_DIM], f32, tag="ps", name=f"ps{c}")
                    for c in range(NB)
                ]
                # Main weight sweep: 8 back-to-back matmuls, one LDWEIGHTS.
                for c in range(NB):
                    nc.tensor.matmul(
                        pss[c],
                        w1t,
                        vt[:, c * M_DIM : (c + 1) * M_DIM],
                        start=True,
                        stop=(halo == 0),
                    )
                # Halo matmul for chunk c closes its accumulation group;
                # chain STT + ACT right behind it so the PSUM bank recycles
                # with minimum latency (the next block's main matmul waits
                # on that STT).
                for c in range(NB):
                    if halo > 0:
                        if c + 1 < NB:
                            nxt = vt[0:halo, (c + 1) * M_DIM : (c + 2) * M_DIM]
                        elif b + 1 in vts:
                            nxt = vts[b + 1][0:halo, 0:M_DIM]
                        else:
                            nxt = vht[:, :]
                        nc.tensor.matmul(pss[c], w2t, nxt, start=False, stop=True)
                    phase = c % 4
                    # Fused (ps * 1.0) * C4row with free-axis accumulation:
                    # one DVE pass gives scalar[w] instead of mult + reduce.
                    sc = work.tile([CHUNK, M_DIM], f32, tag="sc")
                    s = work.tile([CHUNK, 1], f32, tag="s")
                    nc.vector.scalar_tensor_tensor(
                        out=sc,
                        in0=pss[c],
                        scalar=1.0,
                        in1=c4[:, phase, :],
                        op0=mult,
                        op1=mult,
                        accum_out=s,
                    )
                    nc.scalar.activation(
                        out=ot[:, c * M_DIM : (c + 1) * M_DIM],
                        in_=a4[:, phase, :],
                        func=copy_f,
                        scale=s,
                    )
                    # Half-block stores on the GPSIMD SWDGE ring: smoother
                    # store stream and a shorter tail flush, without adding
                    # dispatch work to the busy ACT queue. The last block
                    # stores per-chunk so the final flush is only 128KB.
                    if b == nblk - 1:
                        nc.gpsimd.dma_start(
                            out=o_d[b][:, c * M_DIM : (c + 1) * M_DIM],
                            in_=ot[:, c * M_DIM : (c + 1) * M_DIM],
                        )
                    elif c == NB // 2 - 1:
                        half = NB // 2 * M_DIM
                        nc.gpsimd.dma_start(out=o_d[b][:, 0:half], in_=ot[:, 0:half])
                    elif c == NB - 1:
                        half = NB // 2 * M_DIM
                        nc.gpsimd.dma_start(out=o_d[b][:, half:], in_=ot[:, half:])
                del vts[b]

    nc.finalize()
    return nc


def _get_nc(nblk: int, rank: int) -> bass.Bass:
    key = (nblk, rank)
    if key not in _NC_CACHE:
        _NC_CACHE[key] = build_nc(nblk, rank)
    return _NC_CACHE[key]


def make_band_weights(rank: int, dtype):
    """W1[k,w]=1 iff row k of the chunk is inside window w (w<=k<=w+rank-1);
    W2[k,w]=1 iff row k of the *next* chunk is inside window w."""
    w1 = np.zeros((CHUNK, CHUNK), dtype=dtype)
    for k in range(CHUNK):
        w1[k, max(0, k - (rank - 1)) : k + 1] = 1
    halo = rank - 1
    w2 = np.zeros((max(halo, 1), CHUNK), dtype=dtype)
    for k in range(halo):
        w2[k, CHUNK - halo + k :] = 1
    return w1, w2


def kernel(vec_seq, M, Acoeff, Bbasis, rank):
    global _LAST_RESULTS
    import ml_dtypes

    bf = ml_dtypes.bfloat16
    vec_seq = np.asarray(vec_seq, dtype=np.float32)
    M = np.asarray(M, dtype=np.float32)
    Acoeff = np.asarray(Acoeff, dtype=np.float32)
    Bbasis = np.asarray(Bbasis, dtype=np.float32)
    r = int(rank)
    S, m = vec_seq.shape
    assert m == M_DIM and Bbasis.shape[0] == L_DIM
    assert 1 <= r <= CHUNK

    W = S - r + 1  # number of windows
    ntiles = -(-W // (N_CORES * CHUNK))  # psum tiles per core
    nblk = -(-ntiles // NB)  # DMA blocks per core
    blk_rows = NB * CHUNK

    # Host-side parameter precompute (tiny: 512^3 matmul). The 1/rank
    # window-mean scale is folded into C.
    C = ((Bbasis.astype(np.float64) @ M.astype(np.float64)) / r).astype(np.float32)
    AT = np.ascontiguousarray(Acoeff.T).astype(np.float32)
    # psum tile t uses basis rows j = (128*t .. 128*t+127) % 512; per-core
    # tile index k*nblk*NB + b*NB + c has phase c%4 (nblk*NB ≡ 0 mod 4).
    c4 = np.ascontiguousarray(C.reshape(4, CHUNK, M_DIM)).astype(bf)
    a4 = np.ascontiguousarray(AT.reshape(4, CHUNK, M_DIM)).astype(bf)
    w1, w2 = make_band_weights(r, np.float32)
    w1 = w1.astype(ml_dtypes.float8_e4m3fn)
    w2 = w2.astype(ml_dtypes.float8_e4m3fn)

    # Block-swizzle the whole (padded) sequence once: block b holds rows
    # [b*1024, (b+1)*1024) with chunk row p on partition p:
    #   big[b, p, c*512:(c+1)*512] = vec_seq[b*1024 + c*128 + p]
    total_blocks = N_CORES * nblk
    vp = np.zeros((total_blocks * blk_rows + CHUNK, M_DIM), dtype=np.float32)
    vp[:S] = vec_seq
    big = np.ascontiguousarray(
        vp[: total_blocks * blk_rows]
        .reshape(total_blocks, NB, CHUNK, M_DIM)
        .transpose(0, 2, 1, 3)
        .reshape(total_blocks, CHUNK, BLK)
        .astype(bf)
    )
    halo = r - 1
    # First halo rows past each core's region (unswizzled order).
    vhs = [
        np.ascontiguousarray(
            vp[(k + 1) * nblk * blk_rows : (k + 1) * nblk * blk_rows + max(halo, 1)]
        ).astype(bf)
        for k in range(N_CORES)
    ]

    nc = _get_nc(nblk, r)

    in_maps = []
    for k in range(N_CORES):
        im = {
            "v": big[k * nblk : (k + 1) * nblk],
            "cmat": c4,
            "amat": a4,
            "w1": w1,
        }
        if r > 1:
            im["w2"] = w2
            im["vh"] = vhs[k]
        in_maps.append(im)

    res = run_bass_kernel_spmd(nc, in_maps, core_ids=list(range(N_CORES)))
    _LAST_RESULTS = res
    out = np.concatenate(
        [np.asarray(res.results[k]["o"]) for k in range(N_CORES)], axis=0
    )  # [ncores*nblk, 128, NB*512] bf16
    out = (
        out.reshape(-1, CHUNK, NB, M_DIM)
        .transpose(0, 2, 1, 3)
        .reshape(-1, M_DIM)
        .astype(np.float32)
    )
    return np.ascontiguousarray(out[:W])

